# revision 14
# baseline (speedup 1.0000x reference)
"""Channel-attention (nn_CAttention) Trainium2 kernel.

Full inputs in, full output out. Data-parallel over batch B=8 across 8
NeuronCores (one batch element per core); the small [C,C] projection weight
is replicated (passed pre-transposed, cast to bf16, as weight prep).

Per-core math (b fixed, head n in [0,8), c=256 channels, s=2048 spatial):
  qh, kh, vh = q[b].reshape(8, 256, 2048) etc (contiguous view)
  qn = qh / ||qh||_row ; kn likewise          (l2 norm along s)
  GT[d, c] = sum_s kn[d,s] qn[c,s]            (= attn^T)
  sig = sigmoid(GT)
  out_h[c, s] = sum_d sig[d, c] vh[d, s]
  X[32n+j, q*2048+s] = out_h[8j+q, s]         (head -> original channel layout)
  O = W @ X                                   (1x1 conv projection)

Implementation notes:
 - q/k/v are cast fp32->bf16 during the SWDGE DMA load.
 - The [s,c] transposes of q,k are regular matmuls against diag(1/norm),
   so normalization is fused into the transpose for free.
 - The sigmoid writes its output with a permuted free-dim AP (c_new=8j+q
   stored at offset q*32+j), and the out-matmuls are column-packed with
   tile_position=(0,32*hn) so each head group's rank-32 contribution lands
   at the correct PSUM partition base; X then materializes in SBUF already
   in the original-channel layout the projection needs.
 - Transposes are fused per s-block into the GT accumulation (no full
   QT/KT materialization), group-1 loads are emitted before group-0's
   out-matmul phase so input DMA streams under PE-heavy phases, and the
   projection + output DMA chase group-1's out-chunks per 512-column
   t-range.
"""

import os

os.environ.setdefault("JAX_PLATFORMS", "axon,cpu")

import numpy as np
import ml_dtypes
from contextlib import ExitStack

import concourse.bass as bass
import concourse.tile as tile
from concourse import mybir
from concourse._compat import with_exitstack
from concourse.bass import ts, ds
from concourse.bass_utils import run_bass_kernel_spmd
from concourse.masks import make_identity
from concourse.vector_clock import ScopedClock

B, C, HH, WW = 8, 256, 128, 128
NH = 8
S = (HH * WW) // NH  # 2048
HW = HH * WW  # 16384
EPS = 1e-12

F32 = mybir.dt.float32
BF16 = mybir.dt.bfloat16
AF = mybir.ActivationFunctionType

_MAX_DRAIN_WAITS = 1


def _install_drain_patch():
    """This walrus build rejects >1 sync wait on a CTRL instruction; spread
    the TileContext final-drain waits across chained wait-nops on SP."""

    def _drain_and_barrier_split(self, tick_clock, wait_clock):
        nc = self.nc
        drain_inst = nc.sync.drain()
        wait_clock.add_sem_waits(
            drain_inst.ins, ScopedClock({None: tick_clock.global_clock})
        )
        si = drain_inst.ins.sync_info
        waits = list(si.on_wait) if si is not None else []
        if len(waits) > _MAX_DRAIN_WAITS:
            drain_inst.ins.sync_info = mybir.SyncInfo(
                on_wait=waits[:_MAX_DRAIN_WAITS], on_update=[]
            )
            for i in range(_MAX_DRAIN_WAITS, len(waits), _MAX_DRAIN_WAITS):
                nop = nc.sync.nop(nofuse=True, hint="drain_wait_split")
                nop.ins.sync_info = mybir.SyncInfo(
                    on_wait=waits[i : i + _MAX_DRAIN_WAITS], on_update=[]
                )
        nc.all_engine_barrier()
        assert self.sems is not None
        popped = nc._tile_sem_poison_stack.pop()
        assert popped is self._sem_poison
        nc.clear_and_free_semaphores(list(self.sems.allocated().values()))
        nc.all_engine_barrier()

    tile.TileContext._drain_and_barrier = _drain_and_barrier_split


def _split_excess_waits(nc, max_waits=_MAX_DRAIN_WAITS):
    """This walrus build allows only one sync-wait command per instruction;
    hoist extra waits into nofuse NOPs on the same engine just before."""
    n_split = 0
    for f in nc.m.functions:
        for blk in f.blocks:
            il = blk.instructions
            new = []
            for inst in il:
                si = inst.sync_info
                waits = list(si.on_wait) if si is not None else []
                if len(waits) > max_waits:
                    extra, keep = waits[:-max_waits], waits[-max_waits:]
                    for j in range(0, len(extra), max_waits):
                        nop = mybir.InstNoOp(
                            name=f"{inst.name}-wsplit{j}",
                            sync_info=mybir.SyncInfo(
                                on_wait=extra[j : j + max_waits], on_update=[]
                            ),
                            bass_nofuse=True,
                            engine=inst.engine,
                        )
                        new.append(nop)
                    inst.sync_info = mybir.SyncInfo(
                        on_wait=keep, on_update=list(si.on_update)
                    )
                    n_split += 1
                new.append(inst)
            if len(new) != len(il):
                il[:] = new
    return n_split


def _build_perm():
    """Banded permutation selectors for the PE-permute of a in {2,3}:
    P[(ai,nn)][pp, mu] = 1 iff pp = 32*(2+ai)+j and mu = 32*nn+j."""
    P = np.zeros((128, 8, 128), np.float32)
    for ai in range(2):
        a = 2 + ai
        for nn in range(4):
            for j in range(32):
                P[32 * a + j, ai * 4 + nn, 32 * nn + j] = 1.0
    return np.ascontiguousarray(P.reshape(128, 1024)).astype(ml_dtypes.bfloat16)


def _cattn_consts(ctx: ExitStack, tc: tile.TileContext, wt, perm):
    """One-time constants: identity matrices (f32 for diag builds, bf16 for
    raw transposes), permute selectors, and the transposed projection weight
    (bf16) resident in SBUF."""
    nc = tc.nc
    consts = ctx.enter_context(tc.tile_pool(name="consts", bufs=1))
    ident = consts.tile([128, 128], F32)
    make_identity(nc, ident)
    identb = consts.tile([128, 128], BF16)
    nc.vector.tensor_copy(out=identb, in_=ident)
    wt_sb = consts.tile([128, 2, 256], BF16)
    nc.sync.dma_start(out=wt_sb, in_=wt[:].rearrange("(ch p) o -> p ch o", p=128))
    perm_sb = consts.tile([128, 1024], BF16)
    nc.sync.dma_start(out=perm_sb, in_=perm[:])
    return ident, identb, wt_sb, perm_sb


@with_exitstack
def _cattn_body(
    ctx: ExitStack,
    tc: tile.TileContext,
    q,
    k,
    v,
    ident,
    identb,
    wt_sb,
    perm_sb,
    out,
):
    nc = tc.nc

    natp = ctx.enter_context(tc.tile_pool(name="nat", bufs=3))
    sqs = ctx.enter_context(tc.tile_pool(name="sqs", bufs=2))
    stat = ctx.enter_context(tc.tile_pool(name="stat", bufs=8))
    dpool = ctx.enter_context(tc.tile_pool(name="diag", bufs=2))
    qkt = ctx.enter_context(tc.tile_pool(name="qkt", bufs=3))
    sgp = ctx.enter_context(tc.tile_pool(name="sg", bufs=3))
    vp = ctx.enter_context(tc.tile_pool(name="v", bufs=4))
    xp = ctx.enter_context(tc.tile_pool(name="x", bufs=1))
    ysbp = ctx.enter_context(tc.tile_pool(name="ysb", bufs=32))
    obuf = ctx.enter_context(tc.tile_pool(name="obuf", bufs=2))
    tpsum = ctx.enter_context(tc.tile_pool(name="tpsum", bufs=2, space="PSUM"))
    gpsum = ctx.enter_context(tc.tile_pool(name="gpsum", bufs=2, space="PSUM"))
    ypsum = ctx.enter_context(tc.tile_pool(name="ypsum", bufs=2, space="PSUM"))
    ppsum = ctx.enter_context(tc.tile_pool(name="ppsum", bufs=2, space="PSUM"))

    X = xp.tile([128, 2, HW], BF16)

    # PSUM->SBUF copies can only run on DVE and Act (GPSIMD has no PSUM
    # access); weight DVE 2:1 since Act also owns squares/sigmoids. Pool
    # carries the SBUF->SBUF shifted copies.
    cp_state = [0]
    cp_pattern = (nc.vector, nc.scalar, nc.vector)

    def copy_rr(out_ap, in_ap):
        eng = cp_pattern[cp_state[0] % len(cp_pattern)]
        cp_state[0] += 1
        if eng is nc.scalar:
            eng.copy(out=out_ap, in_=in_ap)
        else:
            eng.tensor_copy(out=out_ap, in_=in_ap)

    # Prefetched q/k loads: emitting a DMA early raises its scheduler
    # priority so input streaming continues under PE-heavy phases.
    qk_loads = [None] * NH

    def ensure_qk(n):
        if n >= NH or qk_loads[n] is not None:
            return
        pair = {}
        for name, src in (("q", q), ("k", k)):
            nat = natp.tile([128, 2, S], BF16, tag=f"{name}nat")
            nc.gpsimd.dma_start(
                out=nat, in_=src[n].rearrange("(a p) s -> p a s", p=128)
            )
            pair[name] = nat
        qk_loads[n] = pair

    def process_head(n):
        """Norms, fused normalized-transpose + GT accumulation, sigmoid.

        Only q is normalized in the transpose (diag matmul); k is transposed
        raw against the identity and 1/||k|| is folded into the sigmoid's
        per-partition scale, so k's transpose has no norm dependency.
        """
        pair = qk_loads[n]
        qk_loads[n] = None
        Dq = {}
        rrk = {}
        for name in ("q", "k"):
            nat = pair[name]
            for ct in range(2):
                ssq = stat.tile([128, 1], F32, tag="ssq")
                scr = sqs.tile([128, S], BF16, tag="scr")
                nc.scalar.activation(
                    out=scr, in_=nat[:, ct], func=AF.Square, accum_out=ssq
                )
                nrm = stat.tile([128, 1], F32, tag="nrm")
                nc.scalar.activation(out=nrm, in_=ssq, func=AF.Sqrt)
                nc.vector.tensor_scalar_max(out=nrm, in0=nrm, scalar1=EPS)
                rr = stat.tile([128, 1], F32, tag="rr")
                nc.vector.reciprocal(out=rr, in_=nrm)
                if name == "q":
                    D = dpool.tile([128, 128], BF16, tag="D")
                    nc.vector.tensor_scalar_mul(out=D, in0=ident, scalar1=rr)
                    Dq[ct] = D
                else:
                    rrk[ct] = rr

        # GT accumulators (one PSUM bank per d-tile), live across s-blocks
        gps = [
            gpsum.tile([128, 256], F32, tag="gps", name=f"gps{i}")
            for i in range(2)
        ]
        for sb in range(16):
            ps = tpsum.tile([128, 512], F32, tag="tps")
            for ct in range(2):
                nc.tensor.matmul(
                    ps[:, ds(ct * 128, 128)],
                    lhsT=pair["q"][:, ct, ts(sb, 128)],
                    rhs=Dq[ct],
                    start=True,
                    stop=True,
                )
                nc.tensor.matmul(
                    ps[:, ds(256 + ct * 128, 128)],
                    lhsT=pair["k"][:, ct, ts(sb, 128)],
                    rhs=identb,
                    start=True,
                    stop=True,
                )
            tsb = qkt.tile([128, 512], BF16, tag="tsb")
            copy_rr(tsb, ps)
            for dt_ in range(2):
                nc.tensor.matmul(
                    gps[dt_],
                    lhsT=tsb[:, ds(256 + dt_ * 128, 128)],
                    rhs=tsb[:, 0:256],
                    start=(sb == 0),
                    stop=(sb == 15),
                )

        sg = sgp.tile([128, 2, 256], BF16, tag="sg")
        for dt_ in range(2):
            # c_new = 8j + q_  stored at offset q_*32 + j
            sig_out = sg[:, dt_].rearrange("p (q j) -> p j q", q=8)
            sig_in = gps[dt_][:].rearrange("p (j q) -> p j q", q=8)
            nc.scalar.activation(
                out=sig_out, in_=sig_in, func=AF.Sigmoid, scale=rrk[dt_]
            )
        return sg

    def load_v(n):
        vt = vp.tile([128, 2, S], BF16, tag="v")
        nc.gpsimd.dma_start(
            out=vt, in_=v[n].rearrange("(a p) s -> p a s", p=128)
        )
        return vt

    def proj_chunk(t0):
        """Projection + output stream for one 512-column t-range."""
        ob = obuf.tile([128, 2, 512], F32, tag="ob")
        for ot in range(2):
            pps = ppsum.tile([128, 512], F32, tag="pps")
            for ch in range(2):
                nc.tensor.matmul(
                    pps,
                    lhsT=wt_sb[:, ch, ts(ot, 128)],
                    rhs=X[:, ch, ds(t0, 512)],
                    start=(ch == 0),
                    stop=(ch == 1),
                )
            copy_rr(ob[:, ot], pps)
        nc.sync.dma_start(
            out=out.rearrange("(o2 p) t -> p o2 t", p=128)[:, :, ds(t0, 512)],
            in_=ob,
        )

    def out_chunk(n, m, sc, sg, vt, ysb_round):
        """Head-layout out-matmul (full M=128) for head n's (m, sc) chunk.
        sg's permuted column layout (col 32q+j = sig[d, 8j+q]) makes
        sg[:, dt, 128m:128m+128] exactly the lhsT whose M-column 32a+j is
        channel cc=8j+(4m+a), so Y PSUM partitions 32a..32a+32 hold X rows
        32(n%4)+j of output segment q=4m+a. The a in {0,1} slices go to X
        via Pool's SBUF->SBUF shifted copies (through the ysb staging tile);
        a in {2,3} are gathered across the four heads of the group by the
        PE-permute in permute_round."""
        g, hn = divmod(n, 4)
        yps = ypsum.tile([128, 512], F32, tag="yps")
        for dt_ in range(2):
            nc.tensor.matmul(
                yps,
                lhsT=sg[:, dt_, ds(128 * m, 128)],
                rhs=vt[:, dt_, ds(512 * sc, 512)],
                start=(dt_ == 0),
                stop=(dt_ == 1),
            )
        ysb = ysbp.tile([128, 512], BF16, tag="ysb")
        copy_rr(ysb, yps)
        for a in range(2):
            t0 = (4 * m + a) * 2048 + 512 * sc
            nc.gpsimd.tensor_copy(
                out=X[32 * hn : 32 * hn + 32, g, ds(t0, 512)],
                in_=ysb[32 * a : 32 * a + 32, :],
            )
        ysb_round[hn] = ysb

    def permute_round(g, m, sc, ysb_round):
        """PE gather of the a in {2,3} slices of all four heads of group g
        into X partitions 32nn+j for segment q=4m+a."""
        for ai in range(2):
            xps = ypsum.tile([128, 512], F32, tag="yps")
            for nn in range(4):
                nc.tensor.matmul(
                    xps,
                    lhsT=perm_sb[:, ds((ai * 4 + nn) * 128, 128)],
                    rhs=ysb_round[nn],
                    start=(nn == 0),
                    stop=(nn == 3),
                )
            t0 = (4 * m + 2 + ai) * 2048 + 512 * sc
            copy_rr(X[:, g, ds(t0, 512)], xps)

    # Heads are processed in order; each head's out-chunks are emitted
    # eagerly right after its sigmoid so copy/PE work spreads across the
    # whole body. The group's last head triggers the PE-permute of each
    # (m, sc); the projection chases head 7's rounds (every X column needs
    # all 8 heads, so head 7 is always the gate).
    rounds = [(m, sc) for m in range(2) for sc in range(4)]
    ysb_rounds = {}
    for n in range(NH):
        g, hn = divmod(n, 4)
        ensure_qk(n)
        ensure_qk(n + 1)
        vt = load_v(n)
        sg = process_head(n)
        for m, sc in rounds:
            ysb_round = ysb_rounds.setdefault((g, m, sc), [None] * 4)
            out_chunk(n, m, sc, sg, vt, ysb_round)
            if hn == 3:
                permute_round(g, m, sc, ysb_round)
                ysb_rounds.pop((g, m, sc))
            if n == NH - 1:
                for a in range(4):
                    proj_chunk((4 * m + a) * 2048 + 512 * sc)


_NC_CACHE = {}


def _build_nc(repeats=1):
    if repeats in _NC_CACHE:
        return _NC_CACHE[repeats]
    _install_drain_patch()
    nc = bass.Bass(num_swdge_queues=4)
    q = nc.declare_dram_parameter("q", [NH, C, S], F32, isOutput=False)
    k = nc.declare_dram_parameter("k", [NH, C, S], F32, isOutput=False)
    v = nc.declare_dram_parameter("v", [NH, C, S], F32, isOutput=False)
    wt = nc.declare_dram_parameter("wt", [C, C], BF16, isOutput=False)
    perm = nc.declare_dram_parameter("perm", [128, 1024], BF16, isOutput=False)
    out = nc.declare_dram_parameter("out", [C, HW], F32, isOutput=True)
    trace_sim = bool(os.environ.get("TRACE_SIM"))
    with tile.TileContext(nc, trace_sim=trace_sim) as tc:
        with ExitStack() as const_ctx:
            ident, identb, wt_sb, perm_sb = _cattn_consts(
                const_ctx, tc, wt, perm
            )
            for _ in range(repeats):
                _cattn_body(
                    tc, q, k, v, ident, identb, wt_sb, perm_sb, out
                )
    _split_excess_waits(nc)
    _NC_CACHE[repeats] = nc
    return nc


LAST_RESULT = None


def kernel(q, k, v, w_proj):
    global LAST_RESULT
    q = np.ascontiguousarray(np.asarray(q, dtype=np.float32))
    k = np.ascontiguousarray(np.asarray(k, dtype=np.float32))
    v = np.ascontiguousarray(np.asarray(v, dtype=np.float32))
    w_proj = np.asarray(w_proj, dtype=np.float32)

    nc = _build_nc(1)
    wt = np.ascontiguousarray(w_proj.T).astype(ml_dtypes.bfloat16)
    perm = _build_perm()
    in_maps = [
        {
            "q": q[b].reshape(NH, C, S),
            "k": k[b].reshape(NH, C, S),
            "v": v[b].reshape(NH, C, S),
            "wt": wt,
            "perm": perm,
        }
        for b in range(B)
    ]
    trace = bool(os.environ.get("BASS_TRACE"))
    res = run_bass_kernel_spmd(nc, in_maps, list(range(B)), trace=trace)
    LAST_RESULT = res
    out = np.stack([np.asarray(res.results[b]["out"]) for b in range(B)])
    return out.reshape(B, C, HH, WW).astype(np.float32)


if __name__ == "__main__":
    rng = np.random.default_rng(0)
    qq = rng.standard_normal((B, C, HH, WW), dtype=np.float32)
    kk = rng.standard_normal((B, C, HH, WW), dtype=np.float32)
    vv = rng.standard_normal((B, C, HH, WW), dtype=np.float32)
    wp = rng.standard_normal((C, C), dtype=np.float32) / np.sqrt(C)
    o = kernel(qq, kk, vv, wp)
    print("out shape:", o.shape, "finite:", np.isfinite(o).all())



# revision 20
# speedup vs baseline: 1.2276x; 1.2276x over previous
"""Channel-attention (nn_CAttention) Trainium2 kernel.

Full inputs in, full output out. Data-parallel over batch B=8 across 8
NeuronCores (one batch element per core); the small [C,C] projection weight
is replicated (passed pre-transposed, cast to bf16, as weight prep).

Per-core math (b fixed, head n in [0,8), c=256 channels, s=2048 spatial):
  qh, kh, vh = q[b].reshape(8, 256, 2048) etc (contiguous view)
  qn = qh / ||qh||_row ; kn likewise          (l2 norm along s)
  GT[d, c] = sum_s kn[d,s] qn[c,s]            (= attn^T)
  sig = sigmoid(GT)
  out_h[c, s] = sum_d sig[d, c] vh[d, s]
  X[32n+j, q*2048+s] = out_h[8j+q, s]         (head -> original channel layout)
  O = W @ X                                   (1x1 conv projection)

Implementation notes:
 - q/k/v are cast fp32->bf16 during the SWDGE DMA load.
 - The [s,c] transposes of q,k are regular matmuls against diag(1/norm),
   so normalization is fused into the transpose for free.
 - The sigmoid writes its output with a permuted free-dim AP (c_new=8j+q
   stored at offset q*32+j), and the out-matmuls are column-packed with
   tile_position=(0,32*hn) so each head group's rank-32 contribution lands
   at the correct PSUM partition base; X then materializes in SBUF already
   in the original-channel layout the projection needs.
 - Transposes are fused per s-block into the GT accumulation (no full
   QT/KT materialization), group-1 loads are emitted before group-0's
   out-matmul phase so input DMA streams under PE-heavy phases, and the
   projection + output DMA chase group-1's out-chunks per 512-column
   t-range.
"""

import os

os.environ.setdefault("JAX_PLATFORMS", "axon,cpu")

import numpy as np
import ml_dtypes
from contextlib import ExitStack

import concourse.bass as bass
import concourse.tile as tile
from concourse import mybir
from concourse._compat import with_exitstack
from concourse.bass import ts, ds
from concourse.bass_utils import run_bass_kernel_spmd
from concourse.masks import make_identity
from concourse.vector_clock import ScopedClock

B, C, HH, WW = 8, 256, 128, 128
NH = 8
S = (HH * WW) // NH  # 2048
HW = HH * WW  # 16384
EPS = 1e-12

F32 = mybir.dt.float32
BF16 = mybir.dt.bfloat16
AF = mybir.ActivationFunctionType

_MAX_DRAIN_WAITS = 1


def _install_drain_patch():
    """This walrus build rejects >1 sync wait on a CTRL instruction; spread
    the TileContext final-drain waits across chained wait-nops on SP."""

    def _drain_and_barrier_split(self, tick_clock, wait_clock):
        nc = self.nc
        drain_inst = nc.sync.drain()
        wait_clock.add_sem_waits(
            drain_inst.ins, ScopedClock({None: tick_clock.global_clock})
        )
        si = drain_inst.ins.sync_info
        waits = list(si.on_wait) if si is not None else []
        if len(waits) > _MAX_DRAIN_WAITS:
            drain_inst.ins.sync_info = mybir.SyncInfo(
                on_wait=waits[:_MAX_DRAIN_WAITS], on_update=[]
            )
            for i in range(_MAX_DRAIN_WAITS, len(waits), _MAX_DRAIN_WAITS):
                nop = nc.sync.nop(nofuse=True, hint="drain_wait_split")
                nop.ins.sync_info = mybir.SyncInfo(
                    on_wait=waits[i : i + _MAX_DRAIN_WAITS], on_update=[]
                )
        nc.all_engine_barrier()
        assert self.sems is not None
        popped = nc._tile_sem_poison_stack.pop()
        assert popped is self._sem_poison
        nc.clear_and_free_semaphores(list(self.sems.allocated().values()))
        nc.all_engine_barrier()

    tile.TileContext._drain_and_barrier = _drain_and_barrier_split


def _split_excess_waits(nc, max_waits=_MAX_DRAIN_WAITS):
    """This walrus build allows only one sync-wait command per instruction;
    hoist extra waits into nofuse NOPs on the same engine just before."""
    n_split = 0
    for f in nc.m.functions:
        for blk in f.blocks:
            il = blk.instructions
            new = []
            for inst in il:
                si = inst.sync_info
                waits = list(si.on_wait) if si is not None else []
                if len(waits) > max_waits:
                    extra, keep = waits[:-max_waits], waits[-max_waits:]
                    for j in range(0, len(extra), max_waits):
                        nop = mybir.InstNoOp(
                            name=f"{inst.name}-wsplit{j}",
                            sync_info=mybir.SyncInfo(
                                on_wait=extra[j : j + max_waits], on_update=[]
                            ),
                            bass_nofuse=True,
                            engine=inst.engine,
                        )
                        new.append(nop)
                    inst.sync_info = mybir.SyncInfo(
                        on_wait=keep, on_update=list(si.on_update)
                    )
                    n_split += 1
                new.append(inst)
            if len(new) != len(il):
                il[:] = new
    return n_split


def _cattn_consts(ctx: ExitStack, tc: tile.TileContext, wt):
    """One-time constants: identity matrices (f32 for diag builds, bf16 for
    raw transposes) and the transposed projection weight (bf16) in SBUF."""
    nc = tc.nc
    consts = ctx.enter_context(tc.tile_pool(name="consts", bufs=1))
    ident = consts.tile([128, 128], F32)
    make_identity(nc, ident)
    identb = consts.tile([128, 128], BF16)
    nc.vector.tensor_copy(out=identb, in_=ident)
    wt_sb = consts.tile([128, 2, 256], BF16)
    nc.sync.dma_start(out=wt_sb, in_=wt[:].rearrange("(ch p) o -> p ch o", p=128))
    return ident, identb, wt_sb


@with_exitstack
def _cattn_body(
    ctx: ExitStack, tc: tile.TileContext, q, k, v, ident, identb, wt_sb, out
):
    nc = tc.nc

    natp = ctx.enter_context(tc.tile_pool(name="nat", bufs=4))
    sqs = ctx.enter_context(tc.tile_pool(name="sqs", bufs=2))
    stat = ctx.enter_context(tc.tile_pool(name="stat", bufs=8))
    dpool = ctx.enter_context(tc.tile_pool(name="diag", bufs=2))
    qkt = ctx.enter_context(tc.tile_pool(name="qkt", bufs=3))
    sgp = ctx.enter_context(tc.tile_pool(name="sg", bufs=3))
    vp = ctx.enter_context(tc.tile_pool(name="v", bufs=4))
    xp = ctx.enter_context(tc.tile_pool(name="x", bufs=1))
    obuf = ctx.enter_context(tc.tile_pool(name="obuf", bufs=3))
    tpsum = ctx.enter_context(tc.tile_pool(name="tpsum", bufs=2, space="PSUM"))
    gpsum = ctx.enter_context(tc.tile_pool(name="gpsum", bufs=2, space="PSUM"))
    ypsum = ctx.enter_context(tc.tile_pool(name="ypsum", bufs=2, space="PSUM"))
    ppsum = ctx.enter_context(tc.tile_pool(name="ppsum", bufs=2, space="PSUM"))

    X = xp.tile([128, 2, HW], BF16)

    # PSUM->SBUF copies can only run on DVE and Act (GPSIMD has no PSUM
    # access, and is ~4x slower than modeled for copies anyway). Measured on
    # HW: DVE ~0.85 ns/col, Act ~0.67 ns/col; Act also owns the squares and
    # sigmoids, so weight DVE 3:2.
    cp_state = [0]
    cp_pattern = (nc.vector, nc.scalar, nc.vector, nc.vector, nc.scalar)

    def copy_rr(out_ap, in_ap):
        eng = cp_pattern[cp_state[0] % len(cp_pattern)]
        cp_state[0] += 1
        if eng is nc.scalar:
            eng.copy(out=out_ap, in_=in_ap)
        else:
            eng.tensor_copy(out=out_ap, in_=in_ap)

    # Prefetched q/k loads: emitting a DMA early raises its scheduler
    # priority so input streaming continues under PE-heavy phases.
    qk_loads = [None] * NH

    def ensure_qk(n):
        if n >= NH or qk_loads[n] is not None:
            return
        pair = {}
        for name, src in (("q", q), ("k", k)):
            nat = natp.tile([128, 2, S], BF16, tag=f"{name}nat")
            nc.gpsimd.dma_start(
                out=nat, in_=src[n].rearrange("(a p) s -> p a s", p=128)
            )
            pair[name] = nat
        qk_loads[n] = pair

    def process_head(n):
        """Norms, fused normalized-transpose + GT accumulation, sigmoid.

        Only q is normalized in the transpose (diag matmul); k is transposed
        raw against the identity and 1/||k|| is folded into the sigmoid's
        per-partition scale, so k's transpose has no norm dependency.
        """
        pair = qk_loads[n]
        qk_loads[n] = None
        Dq = {}
        rrk = {}
        for name in ("q", "k"):
            nat = pair[name]
            for ct in range(2):
                ssq = stat.tile([128, 1], F32, tag="ssq")
                scr = sqs.tile([128, S], BF16, tag="scr")
                nc.scalar.activation(
                    out=scr, in_=nat[:, ct], func=AF.Square, accum_out=ssq
                )
                nrm = stat.tile([128, 1], F32, tag="nrm")
                nc.scalar.activation(out=nrm, in_=ssq, func=AF.Sqrt)
                nc.vector.tensor_scalar_max(out=nrm, in0=nrm, scalar1=EPS)
                rr = stat.tile([128, 1], F32, tag="rr")
                nc.vector.reciprocal(out=rr, in_=nrm)
                if name == "q":
                    D = dpool.tile([128, 128], BF16, tag="D")
                    nc.vector.tensor_scalar_mul(out=D, in0=ident, scalar1=rr)
                    Dq[ct] = D
                else:
                    rrk[ct] = rr

        # GT accumulators (one PSUM bank per d-tile), live across s-blocks
        gps = [
            gpsum.tile([128, 256], F32, tag="gps", name=f"gps{i}")
            for i in range(2)
        ]
        for sb in range(16):
            ps = tpsum.tile([128, 512], F32, tag="tps")
            for ct in range(2):
                nc.tensor.matmul(
                    ps[:, ds(ct * 128, 128)],
                    lhsT=pair["q"][:, ct, ts(sb, 128)],
                    rhs=Dq[ct],
                    start=True,
                    stop=True,
                )
                nc.tensor.matmul(
                    ps[:, ds(256 + ct * 128, 128)],
                    lhsT=pair["k"][:, ct, ts(sb, 128)],
                    rhs=identb,
                    start=True,
                    stop=True,
                )
            tsb = qkt.tile([128, 512], BF16, tag="tsb")
            copy_rr(tsb, ps)
            for dt_ in range(2):
                nc.tensor.matmul(
                    gps[dt_],
                    lhsT=tsb[:, ds(256 + dt_ * 128, 128)],
                    rhs=tsb[:, 0:256],
                    start=(sb == 0),
                    stop=(sb == 15),
                )

        sg = sgp.tile([128, 2, 256], BF16, tag="sg")
        for dt_ in range(2):
            # c_new = 8j + q_  stored at offset q_*32 + j
            sig_out = sg[:, dt_].rearrange("p (q j) -> p j q", q=8)
            sig_in = gps[dt_][:].rearrange("p (j q) -> p j q", q=8)
            nc.scalar.activation(
                out=sig_out, in_=sig_in, func=AF.Sigmoid, scale=rrk[dt_]
            )
        return sg

    def load_v(n):
        vt = vp.tile([128, 2, S], BF16, tag="v")
        nc.gpsimd.dma_start(
            out=vt, in_=v[n].rearrange("(a p) s -> p a s", p=128)
        )
        return vt

    def proj_chunk(t0):
        """Projection + output stream for one 512-column t-range."""
        ob = obuf.tile([128, 2, 512], F32, tag="ob")
        for ot in range(2):
            pps = ppsum.tile([128, 512], F32, tag="pps")
            for ch in range(2):
                nc.tensor.matmul(
                    pps,
                    lhsT=wt_sb[:, ch, ts(ot, 128)],
                    rhs=X[:, ch, ds(t0, 512)],
                    start=(ch == 0),
                    stop=(ch == 1),
                )
            copy_rr(ob[:, ot], pps)
        nc.sync.dma_start(
            out=out.rearrange("(o2 p) t -> p o2 t", p=128)[:, :, ds(t0, 512)],
            in_=ob,
        )

    def out_chunk(n, m, sc, sg, vt):
        """Head-layout out-matmul (full M=128) for head n's (m, sc) chunk,
        scattered into X's original-channel layout by partition-shifted
        PSUM->SBUF copies. sg's permuted column layout (col 32q+j =
        sig[d, 8j+q]) makes sg[:, dt, 128m:128m+128] exactly the lhsT whose
        M-column 32a+j is channel cc=8j+(4m+a), so Y PSUM partitions
        32a..32a+32 hold X rows 32(n%4)+j of output segment q=4m+a."""
        g, hn = divmod(n, 4)
        yps = ypsum.tile([128, 512], F32, tag="yps")
        for dt_ in range(2):
            nc.tensor.matmul(
                yps,
                lhsT=sg[:, dt_, ds(128 * m, 128)],
                rhs=vt[:, dt_, ds(512 * sc, 512)],
                start=(dt_ == 0),
                stop=(dt_ == 1),
            )
        for a in range(4):
            t0 = (4 * m + a) * 2048 + 512 * sc
            copy_rr(
                X[32 * hn : 32 * hn + 32, g, ds(t0, 512)],
                yps[32 * a : 32 * a + 32, :],
            )

    # Heads are processed in order; each head's out-chunks are emitted
    # eagerly right after its sigmoid so copy/PE work spreads across the
    # whole body. The projection chases head 7's rounds (every X column
    # needs all 8 heads, so head 7 is always the gate).
    rounds = [(m, sc) for m in range(2) for sc in range(4)]
    for n in range(NH):
        ensure_qk(n)
        ensure_qk(n + 1)
        vt = load_v(n)
        sg = process_head(n)
        for m, sc in rounds:
            out_chunk(n, m, sc, sg, vt)
            if n == NH - 1:
                for a in range(4):
                    proj_chunk((4 * m + a) * 2048 + 512 * sc)


_NC_CACHE = {}


def _build_nc(repeats=1):
    if repeats in _NC_CACHE:
        return _NC_CACHE[repeats]
    _install_drain_patch()
    nc = bass.Bass(num_swdge_queues=4)
    q = nc.declare_dram_parameter("q", [NH, C, S], F32, isOutput=False)
    k = nc.declare_dram_parameter("k", [NH, C, S], F32, isOutput=False)
    v = nc.declare_dram_parameter("v", [NH, C, S], F32, isOutput=False)
    wt = nc.declare_dram_parameter("wt", [C, C], BF16, isOutput=False)
    out = nc.declare_dram_parameter("out", [C, HW], F32, isOutput=True)
    trace_sim = bool(os.environ.get("TRACE_SIM"))
    with tile.TileContext(nc, trace_sim=trace_sim) as tc:
        with ExitStack() as const_ctx:
            ident, identb, wt_sb = _cattn_consts(const_ctx, tc, wt)
            for _ in range(repeats):
                _cattn_body(tc, q, k, v, ident, identb, wt_sb, out)
    _split_excess_waits(nc)
    _NC_CACHE[repeats] = nc
    return nc


LAST_RESULT = None


def kernel(q, k, v, w_proj):
    global LAST_RESULT
    q = np.ascontiguousarray(np.asarray(q, dtype=np.float32))
    k = np.ascontiguousarray(np.asarray(k, dtype=np.float32))
    v = np.ascontiguousarray(np.asarray(v, dtype=np.float32))
    w_proj = np.asarray(w_proj, dtype=np.float32)

    nc = _build_nc(1)
    wt = np.ascontiguousarray(w_proj.T).astype(ml_dtypes.bfloat16)
    in_maps = [
        {
            "q": q[b].reshape(NH, C, S),
            "k": k[b].reshape(NH, C, S),
            "v": v[b].reshape(NH, C, S),
            "wt": wt,
        }
        for b in range(B)
    ]
    trace = bool(os.environ.get("BASS_TRACE"))
    res = run_bass_kernel_spmd(nc, in_maps, list(range(B)), trace=trace)
    LAST_RESULT = res
    out = np.stack([np.asarray(res.results[b]["out"]) for b in range(B)])
    return out.reshape(B, C, HH, WW).astype(np.float32)


if __name__ == "__main__":
    rng = np.random.default_rng(0)
    qq = rng.standard_normal((B, C, HH, WW), dtype=np.float32)
    kk = rng.standard_normal((B, C, HH, WW), dtype=np.float32)
    vv = rng.standard_normal((B, C, HH, WW), dtype=np.float32)
    wp = rng.standard_normal((C, C), dtype=np.float32) / np.sqrt(C)
    o = kernel(qq, kk, vv, wp)
    print("out shape:", o.shape, "finite:", np.isfinite(o).all())



# revision 22
# speedup vs baseline: 1.4183x; 1.1553x over previous
"""Channel-attention (nn_CAttention) Trainium2 kernel.

Full inputs in, full output out. Data-parallel over batch B=8 across 8
NeuronCores (one batch element per core); the small [C,C] projection weight
is replicated (passed pre-transposed, cast to bf16, as weight prep).

Per-core math (b fixed, head n in [0,8), c=256 channels, s=2048 spatial):
  qh, kh, vh = q[b].reshape(8, 256, 2048) etc (contiguous view)
  qn = qh / ||qh||_row ; kn likewise          (l2 norm along s)
  GT[d, c] = sum_s kn[d,s] qn[c,s]            (= attn^T)
  sig = sigmoid(GT)
  out_h[c, s] = sum_d sig[d, c] vh[d, s]
  X[32n+j, q*2048+s] = out_h[8j+q, s]         (head -> original channel layout)
  O = W @ X                                   (1x1 conv projection)

Implementation notes:
 - q/k/v are cast fp32->bf16 during the SWDGE DMA load.
 - The [s,c] transposes of q,k are regular matmuls against diag(1/norm),
   so normalization is fused into the transpose for free.
 - The sigmoid writes its output with a permuted free-dim AP (c_new=8j+q
   stored at offset q*32+j), and the out-matmuls are column-packed with
   tile_position=(0,32*hn) so each head group's rank-32 contribution lands
   at the correct PSUM partition base; X then materializes in SBUF already
   in the original-channel layout the projection needs.
 - Transposes are fused per s-block into the GT accumulation (no full
   QT/KT materialization), group-1 loads are emitted before group-0's
   out-matmul phase so input DMA streams under PE-heavy phases, and the
   projection + output DMA chase group-1's out-chunks per 512-column
   t-range.
"""

import os

os.environ.setdefault("JAX_PLATFORMS", "axon,cpu")

import numpy as np
import ml_dtypes
from contextlib import ExitStack

import concourse.bass as bass
import concourse.tile as tile
from concourse import mybir
from concourse._compat import with_exitstack
from concourse.bass import ts, ds
from concourse.bass_utils import run_bass_kernel_spmd
from concourse.masks import make_identity
from concourse.vector_clock import ScopedClock

B, C, HH, WW = 8, 256, 128, 128
NH = 8
S = (HH * WW) // NH  # 2048
HW = HH * WW  # 16384
EPS = 1e-12

F32 = mybir.dt.float32
BF16 = mybir.dt.bfloat16
AF = mybir.ActivationFunctionType

_MAX_DRAIN_WAITS = 1


def _install_drain_patch():
    """This walrus build rejects >1 sync wait on a CTRL instruction; spread
    the TileContext final-drain waits across chained wait-nops on SP."""

    def _drain_and_barrier_split(self, tick_clock, wait_clock):
        nc = self.nc
        drain_inst = nc.sync.drain()
        wait_clock.add_sem_waits(
            drain_inst.ins, ScopedClock({None: tick_clock.global_clock})
        )
        si = drain_inst.ins.sync_info
        waits = list(si.on_wait) if si is not None else []
        if len(waits) > _MAX_DRAIN_WAITS:
            drain_inst.ins.sync_info = mybir.SyncInfo(
                on_wait=waits[:_MAX_DRAIN_WAITS], on_update=[]
            )
            for i in range(_MAX_DRAIN_WAITS, len(waits), _MAX_DRAIN_WAITS):
                nop = nc.sync.nop(nofuse=True, hint="drain_wait_split")
                nop.ins.sync_info = mybir.SyncInfo(
                    on_wait=waits[i : i + _MAX_DRAIN_WAITS], on_update=[]
                )
        nc.all_engine_barrier()
        assert self.sems is not None
        popped = nc._tile_sem_poison_stack.pop()
        assert popped is self._sem_poison
        nc.clear_and_free_semaphores(list(self.sems.allocated().values()))
        nc.all_engine_barrier()

    tile.TileContext._drain_and_barrier = _drain_and_barrier_split


def _split_excess_waits(nc, max_waits=_MAX_DRAIN_WAITS):
    """This walrus build allows only one sync-wait command per instruction;
    hoist extra waits into nofuse NOPs on the same engine just before."""
    n_split = 0
    for f in nc.m.functions:
        for blk in f.blocks:
            il = blk.instructions
            new = []
            for inst in il:
                si = inst.sync_info
                waits = list(si.on_wait) if si is not None else []
                if len(waits) > max_waits:
                    extra, keep = waits[:-max_waits], waits[-max_waits:]
                    for j in range(0, len(extra), max_waits):
                        nop = mybir.InstNoOp(
                            name=f"{inst.name}-wsplit{j}",
                            sync_info=mybir.SyncInfo(
                                on_wait=extra[j : j + max_waits], on_update=[]
                            ),
                            bass_nofuse=True,
                            engine=inst.engine,
                        )
                        new.append(nop)
                    inst.sync_info = mybir.SyncInfo(
                        on_wait=keep, on_update=list(si.on_update)
                    )
                    n_split += 1
                new.append(inst)
            if len(new) != len(il):
                il[:] = new
    return n_split


def _cattn_consts(ctx: ExitStack, tc: tile.TileContext, wt):
    """One-time constants: identity matrices (f32 for diag builds, bf16 for
    raw transposes) and the transposed projection weight (bf16) in SBUF."""
    nc = tc.nc
    consts = ctx.enter_context(tc.tile_pool(name="consts", bufs=1))
    ident = consts.tile([128, 128], F32)
    make_identity(nc, ident)
    identb = consts.tile([128, 128], BF16)
    nc.vector.tensor_copy(out=identb, in_=ident)
    wt_sb = consts.tile([128, 2, 256], BF16)
    nc.sync.dma_start(out=wt_sb, in_=wt[:].rearrange("(ch p) o -> p ch o", p=128))
    return ident, identb, wt_sb


def _cattn_pools(ctx: ExitStack, tc: tile.TileContext):
    """Pools are created once and shared across repeats so the tile rings
    rotate across body boundaries (cross-iteration pipelining)."""
    p = {}
    p["natp"] = ctx.enter_context(tc.tile_pool(name="nat", bufs=4))
    p["sqs"] = ctx.enter_context(tc.tile_pool(name="sqs", bufs=2))
    p["stat"] = ctx.enter_context(tc.tile_pool(name="stat", bufs=8))
    p["dpool"] = ctx.enter_context(tc.tile_pool(name="diag", bufs=2))
    p["qkt"] = ctx.enter_context(tc.tile_pool(name="qkt", bufs=3))
    p["sgp"] = ctx.enter_context(tc.tile_pool(name="sg", bufs=3))
    p["vp"] = ctx.enter_context(tc.tile_pool(name="v", bufs=4))
    p["xp"] = ctx.enter_context(tc.tile_pool(name="x", bufs=1))
    p["obuf"] = ctx.enter_context(tc.tile_pool(name="obuf", bufs=3))
    p["tpsum"] = ctx.enter_context(
        tc.tile_pool(name="tpsum", bufs=2, space="PSUM")
    )
    p["gpsum"] = ctx.enter_context(
        tc.tile_pool(name="gpsum", bufs=2, space="PSUM")
    )
    p["ypsum"] = ctx.enter_context(
        tc.tile_pool(name="ypsum", bufs=2, space="PSUM")
    )
    p["ppsum"] = ctx.enter_context(
        tc.tile_pool(name="ppsum", bufs=2, space="PSUM")
    )
    return p


def _cattn_body(tc: tile.TileContext, p, q, k, v, ident, identb, wt_sb, out):
    nc = tc.nc

    natp = p["natp"]
    sqs = p["sqs"]
    stat = p["stat"]
    dpool = p["dpool"]
    qkt = p["qkt"]
    sgp = p["sgp"]
    vp = p["vp"]
    obuf = p["obuf"]
    tpsum = p["tpsum"]
    gpsum = p["gpsum"]
    ypsum = p["ypsum"]
    ppsum = p["ppsum"]

    X = p["xp"].tile([128, 2, HW], BF16, tag="X")

    # PSUM->SBUF copies can only run on DVE and Act (GPSIMD has no PSUM
    # access, and is ~4x slower than modeled for copies anyway). Measured on
    # HW: DVE ~0.85 ns/col, Act ~0.67 ns/col; Act also owns the squares and
    # sigmoids, so weight DVE 3:2.
    cp_state = [0]
    cp_pattern = (nc.vector, nc.scalar, nc.vector, nc.vector, nc.scalar)

    def copy_rr(out_ap, in_ap):
        eng = cp_pattern[cp_state[0] % len(cp_pattern)]
        cp_state[0] += 1
        if eng is nc.scalar:
            eng.copy(out=out_ap, in_=in_ap)
        else:
            eng.tensor_copy(out=out_ap, in_=in_ap)

    # Prefetched q/k loads: emitting a DMA early raises its scheduler
    # priority so input streaming continues under PE-heavy phases.
    qk_loads = [None] * NH

    def ensure_qk(n):
        if n >= NH or qk_loads[n] is not None:
            return
        pair = {}
        for name, src in (("q", q), ("k", k)):
            nat = natp.tile([128, 2, S], BF16, tag=f"{name}nat")
            nc.gpsimd.dma_start(
                out=nat, in_=src[n].rearrange("(a p) s -> p a s", p=128)
            )
            pair[name] = nat
        qk_loads[n] = pair

    def process_head(n):
        """Norms, fused normalized-transpose + GT accumulation, sigmoid.

        Only q is normalized in the transpose (diag matmul); k is transposed
        raw against the identity and 1/||k|| is folded into the sigmoid's
        per-partition scale, so k's transpose has no norm dependency.
        """
        pair = qk_loads[n]
        qk_loads[n] = None
        Dq = {}
        rrk = {}
        for name in ("q", "k"):
            nat = pair[name]
            for ct in range(2):
                ssq = stat.tile([128, 1], F32, tag="ssq")
                scr = sqs.tile([128, S], BF16, tag="scr")
                nc.scalar.activation(
                    out=scr, in_=nat[:, ct], func=AF.Square, accum_out=ssq
                )
                nrm = stat.tile([128, 1], F32, tag="nrm")
                nc.scalar.activation(out=nrm, in_=ssq, func=AF.Sqrt)
                nc.vector.tensor_scalar_max(out=nrm, in0=nrm, scalar1=EPS)
                rr = stat.tile([128, 1], F32, tag="rr")
                nc.vector.reciprocal(out=rr, in_=nrm)
                if name == "q":
                    D = dpool.tile([128, 128], BF16, tag="D")
                    nc.vector.tensor_scalar_mul(out=D, in0=ident, scalar1=rr)
                    Dq[ct] = D
                else:
                    rrk[ct] = rr

        # GT accumulators (one PSUM bank per d-tile), live across s-blocks
        gps = [
            gpsum.tile([128, 256], F32, tag="gps", name=f"gps{i}")
            for i in range(2)
        ]
        for sb in range(16):
            ps = tpsum.tile([128, 512], F32, tag="tps")
            for ct in range(2):
                nc.tensor.matmul(
                    ps[:, ds(ct * 128, 128)],
                    lhsT=pair["q"][:, ct, ts(sb, 128)],
                    rhs=Dq[ct],
                    start=True,
                    stop=True,
                )
                nc.tensor.matmul(
                    ps[:, ds(256 + ct * 128, 128)],
                    lhsT=pair["k"][:, ct, ts(sb, 128)],
                    rhs=identb,
                    start=True,
                    stop=True,
                )
            tsb = qkt.tile([128, 512], BF16, tag="tsb")
            copy_rr(tsb, ps)
            for dt_ in range(2):
                nc.tensor.matmul(
                    gps[dt_],
                    lhsT=tsb[:, ds(256 + dt_ * 128, 128)],
                    rhs=tsb[:, 0:256],
                    start=(sb == 0),
                    stop=(sb == 15),
                )

        sg = sgp.tile([128, 2, 256], BF16, tag="sg")
        for dt_ in range(2):
            # c_new = 8j + q_  stored at offset q_*32 + j
            sig_out = sg[:, dt_].rearrange("p (q j) -> p j q", q=8)
            sig_in = gps[dt_][:].rearrange("p (j q) -> p j q", q=8)
            nc.scalar.activation(
                out=sig_out, in_=sig_in, func=AF.Sigmoid, scale=rrk[dt_]
            )
        return sg

    def load_v(n):
        vt = vp.tile([128, 2, S], BF16, tag="v")
        nc.gpsimd.dma_start(
            out=vt, in_=v[n].rearrange("(a p) s -> p a s", p=128)
        )
        return vt

    def proj_chunk(t0):
        """Projection + output stream for one 512-column t-range."""
        ob = obuf.tile([128, 2, 512], F32, tag="ob")
        for ot in range(2):
            pps = ppsum.tile([128, 512], F32, tag="pps")
            for ch in range(2):
                nc.tensor.matmul(
                    pps,
                    lhsT=wt_sb[:, ch, ts(ot, 128)],
                    rhs=X[:, ch, ds(t0, 512)],
                    start=(ch == 0),
                    stop=(ch == 1),
                )
            copy_rr(ob[:, ot], pps)
        nc.sync.dma_start(
            out=out.rearrange("(o2 p) t -> p o2 t", p=128)[:, :, ds(t0, 512)],
            in_=ob,
        )

    def out_chunk(n, m, sc, sg, vt):
        """Head-layout out-matmul (full M=128) for head n's (m, sc) chunk,
        scattered into X's original-channel layout by partition-shifted
        PSUM->SBUF copies. sg's permuted column layout (col 32q+j =
        sig[d, 8j+q]) makes sg[:, dt, 128m:128m+128] exactly the lhsT whose
        M-column 32a+j is channel cc=8j+(4m+a), so Y PSUM partitions
        32a..32a+32 hold X rows 32(n%4)+j of output segment q=4m+a."""
        g, hn = divmod(n, 4)
        yps = ypsum.tile([128, 512], F32, tag="yps")
        for dt_ in range(2):
            nc.tensor.matmul(
                yps,
                lhsT=sg[:, dt_, ds(128 * m, 128)],
                rhs=vt[:, dt_, ds(512 * sc, 512)],
                start=(dt_ == 0),
                stop=(dt_ == 1),
            )
        for a in range(4):
            t0 = (4 * m + a) * 2048 + 512 * sc
            copy_rr(
                X[32 * hn : 32 * hn + 32, g, ds(t0, 512)],
                yps[32 * a : 32 * a + 32, :],
            )

    # Heads are processed in order; each head's out-chunks are emitted
    # eagerly right after its sigmoid so copy/PE work spreads across the
    # whole body. The projection chases head 7's rounds (every X column
    # needs all 8 heads, so head 7 is always the gate).
    rounds = [(m, sc) for m in range(2) for sc in range(4)]
    for n in range(NH):
        ensure_qk(n)
        ensure_qk(n + 1)
        vt = load_v(n)
        sg = process_head(n)
        for m, sc in rounds:
            out_chunk(n, m, sc, sg, vt)
            if n == NH - 1:
                for a in range(4):
                    proj_chunk((4 * m + a) * 2048 + 512 * sc)


_NC_CACHE = {}


def _build_nc(repeats=1):
    if repeats in _NC_CACHE:
        return _NC_CACHE[repeats]
    _install_drain_patch()
    nc = bass.Bass(num_swdge_queues=4)
    q = nc.declare_dram_parameter("q", [NH, C, S], F32, isOutput=False)
    k = nc.declare_dram_parameter("k", [NH, C, S], F32, isOutput=False)
    v = nc.declare_dram_parameter("v", [NH, C, S], F32, isOutput=False)
    wt = nc.declare_dram_parameter("wt", [C, C], BF16, isOutput=False)
    out = nc.declare_dram_parameter("out", [C, HW], F32, isOutput=True)
    trace_sim = bool(os.environ.get("TRACE_SIM"))
    with tile.TileContext(nc, trace_sim=trace_sim) as tc:
        with ExitStack() as const_ctx:
            ident, identb, wt_sb = _cattn_consts(const_ctx, tc, wt)
            pools = _cattn_pools(const_ctx, tc)
            for _ in range(repeats):
                _cattn_body(tc, pools, q, k, v, ident, identb, wt_sb, out)
    _split_excess_waits(nc)
    _NC_CACHE[repeats] = nc
    return nc


LAST_RESULT = None


def kernel(q, k, v, w_proj):
    global LAST_RESULT
    q = np.ascontiguousarray(np.asarray(q, dtype=np.float32))
    k = np.ascontiguousarray(np.asarray(k, dtype=np.float32))
    v = np.ascontiguousarray(np.asarray(v, dtype=np.float32))
    w_proj = np.asarray(w_proj, dtype=np.float32)

    nc = _build_nc(1)
    wt = np.ascontiguousarray(w_proj.T).astype(ml_dtypes.bfloat16)
    in_maps = [
        {
            "q": q[b].reshape(NH, C, S),
            "k": k[b].reshape(NH, C, S),
            "v": v[b].reshape(NH, C, S),
            "wt": wt,
        }
        for b in range(B)
    ]
    trace = bool(os.environ.get("BASS_TRACE"))
    res = run_bass_kernel_spmd(nc, in_maps, list(range(B)), trace=trace)
    LAST_RESULT = res
    out = np.stack([np.asarray(res.results[b]["out"]) for b in range(B)])
    return out.reshape(B, C, HH, WW).astype(np.float32)


if __name__ == "__main__":
    rng = np.random.default_rng(0)
    qq = rng.standard_normal((B, C, HH, WW), dtype=np.float32)
    kk = rng.standard_normal((B, C, HH, WW), dtype=np.float32)
    vv = rng.standard_normal((B, C, HH, WW), dtype=np.float32)
    wp = rng.standard_normal((C, C), dtype=np.float32) / np.sqrt(C)
    o = kernel(qq, kk, vv, wp)
    print("out shape:", o.shape, "finite:", np.isfinite(o).all())

